# revision 7
# baseline (speedup 1.0000x reference)
"""Trainium2 Bass kernel for nn_CollisionAccuracy (1-NN collision count).

Problem: B=4 batches, Nq=8192 query points, Na=6890 anchor points (3D).
For each query: find exact nearest anchor (argmin of squared distance),
then collision(q) = (||q - a_nn|| <= 0.5) and ((q - a_nn) . n_nn < 0).
Output: per-batch collision counts [B, 1] float32.

Formulation used on device (avoids any argmin-index/gather):
    d2(q,a) = ||q||^2 - 2 q.a + ||a||^2            (PE matmul, K-packed)
    s(q,a)  = q.n_a - a.n_a                        (PE matmul, K-packed)
      (note: s = (q - a).n_a, i.e. the collision dot product IF a were the NN)
    m1(q) = min_a d2(q,a)
    m2(q) = min_a (d2(q,a) + relu(BIGSCALE * s(q,a)))   (min over s<0 anchors)
    collision(q) = (m2 == m1) && (m1 <= 0.25)
Exactness: relu term is 0 exactly for s<0 anchors, so if the true NN has
s<0 then m2 == m1 bitwise; otherwise m2 > m1 (pushed by BIGSCALE*s).

Precision: matmuls run in fp16 with hi/lo splitting (q = qh + ql etc.),
K-packed so all cross terms accumulate in fp32 PSUM -> ~1e-9 accurate d2.

Sharding: 8 cores; core c handles batch c//2, query half c%2 (4096 queries).
Each core scans all (padded) anchors of its batch. Host sums per-query
collision flags.
"""

import os
import numpy as np

import concourse.bass as bass
import concourse.tile as tile
from concourse import bacc, mybir
from concourse.bass_utils import run_bass_kernel_spmd

# ---- problem constants (hardcoded per contract) ----
B, NQ, NA = 4, 8192, 6890
NCORES = 8
QPC = NQ // 2            # queries per core
PT = 128                 # partitions (queries per tile)
NQT = QPC // PT          # query tiles per core = 32
CHUNK = 512              # anchors per matmul (one PSUM bank of fp32)
GROUP = 1024             # anchors per DVE reduce group (2 PSUM banks)
NAP = 7168               # padded anchor count (14 * 512)
NGROUPS = NAP // GROUP   # 7

K_D2 = 17                # d2 matmul contraction rows (partitions 0..16)
K_S = 14                 # s matmul contraction rows (partitions 32..45)
S_BASE = 32              # matmul base partition must be 0/32/64
KTOT = S_BASE + K_S      # 46 rows in the combined lhs/rhs tensors

MAX_D2 = 0.25            # max_dist^2
BIGSCALE = 1.0e6         # relu scale pushing s>=0 anchors out of m2

LAST_RESULT = None       # BassKernelResults of the most recent run (for test.py)


def _split16(x32):
    """Split fp32 array into (hi, lo) fp16 pair: hi + lo ~= x32 (~2^-22)."""
    x32 = np.ascontiguousarray(x32, dtype=np.float32)
    hi = x32.astype(np.float16)
    lo = (x32 - hi.astype(np.float32)).astype(np.float16)
    return hi, lo


def _split16_3(x32):
    """3-way fp16 split (~2^-33)."""
    x32 = np.ascontiguousarray(x32, dtype=np.float32)
    hi = x32.astype(np.float16)
    r = x32 - hi.astype(np.float32)
    mid = r.astype(np.float16)
    lo = (r - mid.astype(np.float32)).astype(np.float16)
    return hi, mid, lo


def _build_core_arrays(q, a, n):
    """Build per-core lhs [KTOT, QPC] and rhs [KTOT, NAP] fp16 arrays.

    q: [QPC, 3] queries for this core; a: [NA, 3], n: [NA, 3] anchors/normals.
    """
    q = q.astype(np.float32)
    a = a.astype(np.float32)
    n = n.astype(np.float32)

    qh, ql = _split16(q)                       # [QPC, 3]
    m2qh, m2ql = _split16(-2.0 * q)            # exact -2*q split
    q2 = np.sum(q * q, axis=1)                 # [QPC]
    q2h, q2l = _split16(q2)

    ah, al = _split16(a)                       # [NA, 3]
    a2 = np.sum(a.astype(np.float64) * a, axis=1).astype(np.float32)
    a2h, a2m, a2lo = _split16_3(a2)
    nh, nl = _split16(n)
    c = np.sum(a.astype(np.float64) * n, axis=1).astype(np.float32)  # a.n
    nch, ncl = _split16(-c)

    ones_q = np.ones(QPC, np.float16)
    ones_a = np.ones(NA, np.float16)

    # lhs rows (stationary operand, one column per query)
    lhs = np.zeros((KTOT, QPC), np.float16)
    # d2 rows 0..16: coord cross terms (qh*ah, qh*al, ql*ah, ql*al), q2, ones
    lhs[0:3] = m2qh.T
    lhs[3:6] = m2qh.T
    lhs[6:9] = m2ql.T
    lhs[9:12] = m2ql.T
    lhs[12] = q2h
    lhs[13] = q2l
    lhs[14] = ones_q
    lhs[15] = ones_q
    lhs[16] = ones_q
    # s rows 17..30: (qh*nh, qh*nl, ql*nh, ql*nl), ones*(-c hi/lo)
    lhs[32:35] = qh.T
    lhs[35:38] = qh.T
    lhs[38:41] = ql.T
    lhs[41:44] = ql.T
    lhs[44] = ones_q
    lhs[45] = ones_q

    rhs = np.zeros((KTOT, NAP), np.float16)
    rhs[0:3, :NA] = ah.T
    rhs[3:6, :NA] = al.T
    rhs[6:9, :NA] = ah.T
    rhs[9:12, :NA] = al.T
    rhs[12, :NA] = ones_a
    rhs[13, :NA] = ones_a
    rhs[14, :NA] = a2h
    rhs[15, :NA] = a2m
    rhs[16, :NA] = a2lo
    rhs[32:35, :NA] = nh.T
    rhs[35:38, :NA] = nl.T
    rhs[38:41, :NA] = nh.T
    rhs[41:44, :NA] = nl.T
    rhs[44, :NA] = nch
    rhs[45, :NA] = ncl
    # padding anchors: d2 = q2 + 60000 (never the min), s = 0
    rhs[14, NA:] = np.float16(60000.0)
    return lhs, rhs


def _build_program():
    """Build the Bass/Tile program (same NEFF for all 8 cores)."""
    from contextlib import ExitStack

    nc = bacc.Bacc("TRN2", target_bir_lowering=False, debug=False)
    f16, f32 = mybir.dt.float16, mybir.dt.float32

    lhs_d = nc.dram_tensor("lhs", [KTOT, QPC], f16, kind="ExternalInput")
    rhs_d = nc.dram_tensor("rhs", [KTOT, NAP], f16, kind="ExternalInput")
    flags_d = nc.dram_tensor("flags", [PT, NQT], f32, kind="ExternalOutput")
    m1_d = nc.dram_tensor("m1", [PT, NQT], f32, kind="ExternalOutput")
    m2_d = nc.dram_tensor("m2", [PT, NQT], f32, kind="ExternalOutput")

    with tile.TileContext(nc) as tc, ExitStack() as ctx:
        singles = ctx.enter_context(tc.tile_pool(name="singles", bufs=1))
        psum_d2 = ctx.enter_context(tc.tile_pool(name="psum_d2", bufs=2, space="PSUM"))
        psum_s = ctx.enter_context(tc.tile_pool(name="psum_s", bufs=2, space="PSUM"))
        work = ctx.enter_context(tc.tile_pool(name="work", bufs=3))
        stats = ctx.enter_context(tc.tile_pool(name="stats", bufs=3))

        lhs_sb = singles.tile([KTOT, QPC], f16)
        nc.sync.dma_start(out=lhs_sb[:, :], in_=lhs_d[:, :])
        rhs_sb = singles.tile([KTOT, NAP], f16)
        nc.sync.dma_start(out=rhs_sb[:, :], in_=rhs_d[:, :])

        flags_sb = singles.tile([PT, NQT], f32)
        m1_sb = singles.tile([PT, NQT], f32)
        m2_sb = singles.tile([PT, NQT], f32)

        for t in range(NQT):
            qcol = t * PT
            m1p = stats.tile([PT, NGROUPS], f32, tag="m1p")
            m2p = stats.tile([PT, NGROUPS], f32, tag="m2p")
            for g in range(NGROUPS):
                d2 = psum_d2.tile([PT, GROUP], f32, tag="d2")
                s = psum_s.tile([PT, GROUP], f32, tag="s")
                for h in range(GROUP // CHUNK):
                    acol = g * GROUP + h * CHUNK
                    nc.tensor.matmul(
                        d2[:, h * CHUNK:(h + 1) * CHUNK],
                        lhsT=lhs_sb[0:K_D2, qcol:qcol + PT],
                        rhs=rhs_sb[0:K_D2, acol:acol + CHUNK],
                        start=True, stop=True,
                    )
                    nc.tensor.matmul(
                        s[:, h * CHUNK:(h + 1) * CHUNK],
                        lhsT=lhs_sb[S_BASE:KTOT, qcol:qcol + PT],
                        rhs=rhs_sb[S_BASE:KTOT, acol:acol + CHUNK],
                        start=True, stop=True,
                    )
                mask = work.tile([PT, GROUP], f32, tag="mask")
                nc.scalar.activation(
                    out=mask[:, :], in_=s[:, :],
                    func=mybir.ActivationFunctionType.Relu, scale=BIGSCALE,
                )
                masked = work.tile([PT, GROUP], f32, tag="masked")
                nc.vector.tensor_tensor(
                    masked[:, :], d2[:, :], mask[:, :], mybir.AluOpType.add,
                )
                nc.vector.tensor_reduce(
                    out=m2p[:, g:g + 1], in_=masked[:, :],
                    axis=mybir.AxisListType.X, op=mybir.AluOpType.min,
                )
                nc.vector.tensor_reduce(
                    out=m1p[:, g:g + 1], in_=d2[:, :],
                    axis=mybir.AxisListType.X, op=mybir.AluOpType.min,
                )
            nc.vector.tensor_reduce(
                out=m1_sb[:, t:t + 1], in_=m1p[:, :],
                axis=mybir.AxisListType.X, op=mybir.AluOpType.min,
            )
            nc.vector.tensor_reduce(
                out=m2_sb[:, t:t + 1], in_=m2p[:, :],
                axis=mybir.AxisListType.X, op=mybir.AluOpType.min,
            )
            eq = stats.tile([PT, 1], f32, tag="eq")
            nc.vector.tensor_tensor(
                eq[:, :], m2_sb[:, t:t + 1], m1_sb[:, t:t + 1],
                mybir.AluOpType.is_equal,
            )
            # flags = (m1 <= 0.25) * eq
            nc.vector.scalar_tensor_tensor(
                out=flags_sb[:, t:t + 1], in0=m1_sb[:, t:t + 1], scalar=MAX_D2,
                in1=eq[:, :],
                op0=mybir.AluOpType.is_le, op1=mybir.AluOpType.mult,
            )
        nc.sync.dma_start(out=flags_d[:, :], in_=flags_sb[:, :])
        nc.sync.dma_start(out=m1_d[:, :], in_=m1_sb[:, :])
        nc.sync.dma_start(out=m2_d[:, :], in_=m2_sb[:, :])
    nc.compile()
    return nc


_PROGRAM_CACHE = None
LAST_TIMES = None        # per-execution wall seconds of the most recent run


def _run_pjrt_timed(nc, in_maps, repeats=1):
    """Run the Bass program on the 8 axon cores via PJRT (mirror of
    bass2jax.run_bass_via_pjrt multi-core path), jitting once and executing
    `repeats` times so steady-state per-execution wall time can be measured.

    Returns (per_core_results, wall_times_seconds).
    """
    import time

    import jax
    from jax.experimental.shard_map import shard_map
    from jax.sharding import Mesh, PartitionSpec

    from concourse import mybir as _mybir
    from concourse.bass2jax import (
        _bass_exec_p,
        install_neuronx_cc_hook,
        partition_id_tensor,
    )

    install_neuronx_cc_hook()

    n_cores = len(in_maps)
    partition_name = nc.partition_id_tensor.name if nc.partition_id_tensor else None

    in_names, out_names, out_avals, zero_outs = [], [], [], []
    for alloc in nc.m.functions[0].allocations:
        if not isinstance(alloc, _mybir.MemoryLocationSet):
            continue
        name = alloc.memorylocations[0].name
        if alloc.kind == "ExternalInput":
            if name != partition_name:
                in_names.append(name)
        elif alloc.kind == "ExternalOutput":
            out_names.append(name)
            shape = tuple(alloc.tensor_shape)
            dtype = _mybir.dt.np(alloc.dtype)
            out_avals.append(jax.core.ShapedArray(shape, dtype))
            zero_outs.append(np.zeros(shape, dtype))
    n_params = len(in_names)
    n_outs = len(out_avals)
    all_in_names = list(in_names) + list(out_names)
    if partition_name is not None:
        all_in_names.append(partition_name)

    donate = tuple(range(n_params, n_params + n_outs))

    def _body(*args):
        operands = list(args)
        if partition_name is not None:
            operands.append(partition_id_tensor())
        outs = _bass_exec_p.bind(
            *operands,
            out_avals=tuple(out_avals),
            in_names=tuple(all_in_names),
            out_names=tuple(out_names),
            lowering_input_output_aliases=(),
            sim_require_finite=True,
            sim_require_nnan=True,
            nc=nc,
        )
        return tuple(outs)

    devices = jax.devices()[:n_cores]
    mesh = Mesh(np.asarray(devices), ("core",))
    in_specs = (PartitionSpec("core"),) * (n_params + n_outs)
    out_specs = (PartitionSpec("core"),) * n_outs
    sharded = jax.jit(
        shard_map(_body, mesh=mesh, in_specs=in_specs, out_specs=out_specs,
                  check_rep=False),
        donate_argnums=donate, keep_unused=True,
    )
    concat_in = [
        np.concatenate([np.asarray(in_maps[c][name]) for c in range(n_cores)], axis=0)
        for name in in_names
    ]

    times = []
    out_arrs = None
    for _ in range(max(1, repeats)):
        zeros = [np.zeros((n_cores * z.shape[0], *z.shape[1:]), z.dtype)
                 for z in zero_outs]
        t0 = time.perf_counter()
        out_arrs = sharded(*concat_in, *zeros)
        jax.block_until_ready(out_arrs)
        times.append(time.perf_counter() - t0)

    results = [
        {name: np.asarray(out_arrs[i]).reshape(n_cores, *out_avals[i].shape)[c]
         for i, name in enumerate(out_names)}
        for c in range(n_cores)
    ]
    return results, times


def _prepare_in_maps(query_mesh, anchor_mesh, anchor_normals):
    in_maps = []
    for c in range(NCORES):
        b = c // 2
        half = c % 2
        q = query_mesh[b, half * QPC:(half + 1) * QPC]
        lhs, rhs = _build_core_arrays(q, anchor_mesh[b], anchor_normals[b])
        in_maps.append({"lhs": lhs, "rhs": rhs})
    return in_maps


def _counts_from_results(results):
    counts = np.zeros((B, 1), np.float64)
    for c in range(NCORES):
        counts[c // 2, 0] += results[c]["flags"].sum(dtype=np.float64)
    return counts.astype(np.float32)


def kernel(query_mesh, anchor_mesh, anchor_normals, repeats=1):
    global LAST_RESULT, LAST_TIMES, _PROGRAM_CACHE
    query_mesh = np.asarray(query_mesh, dtype=np.float32)
    anchor_mesh = np.asarray(anchor_mesh, dtype=np.float32)
    anchor_normals = np.asarray(anchor_normals, dtype=np.float32)

    in_maps = _prepare_in_maps(query_mesh, anchor_mesh, anchor_normals)
    if _PROGRAM_CACHE is None:
        _PROGRAM_CACHE = _build_program()
    results, times = _run_pjrt_timed(_PROGRAM_CACHE, in_maps, repeats=repeats)
    LAST_RESULT = results
    LAST_TIMES = times
    return _counts_from_results(results)


if __name__ == "__main__":
    rng = np.random.default_rng(0)
    q = rng.standard_normal((B, NQ, 3), np.float32)
    a = rng.standard_normal((B, NA, 3), np.float32)
    n = rng.standard_normal((B, NA, 3), np.float32)
    n /= np.linalg.norm(n, axis=-1, keepdims=True)
    print(kernel(q, a, n))
